# revision 34
# baseline (speedup 1.0000x reference)
"""LRFGraphConv Trainium2 kernel.

Math: for each vertex i with neighbors N(i) (directed edge list, src=center):
    out[i] = ((sum_{j in N(i)} verts[j] - deg_i * verts[i]) @ lrf[i]) @ W.T + maxN * b

The neighbor-sum commutes with the per-center rotation and GEMM, so the
per-edge work collapses to a segment-sum of neighbor coordinates.  The
rotation and GEMM fuse into a single tensor-engine contraction over the 9
(j,k) pairs of u[i,(j,k)] = t[i,j]*lrf[i,j,k] against Wrep[(j,k),n] = W[n,k],
plus a constant-1 row carrying the maxN*b bias.  u uses 16 slots per vertex
(9 real + bias + 6 pad) so GEMM halves can anchor at partition 0/64.

Sharding: vertices are partitioned contiguously across 8 cores (6250 each).
The host buckets directed edges by owner of src, builds a per-core padded
neighbor table (NP slots, zero padded), and gathers the halo neighbor
coordinates into it (the "halo exchange" done at shard time).  Each core runs
the same NEFF on its own shard over pipelined chunks of up to 8 vertex
tiles:
  DVE:    20-slot reduce + part of the PSUM drain
  Pool:   u = t*lrf broadcast multiply
  PE:     transpose + GEMM (fp16)
  Act:    uT copy + the rest of the PSUM drain
  DMA issue: Sync (neighbor chunks + stores), Act (aux/w/overflow)
No collectives.
"""

import os
import sys

sys.path.insert(0, "/opt/trn_rl_repo")

import numpy as np
import ml_dtypes

import concourse.bass as bass
import concourse.bacc as bacc
import concourse.tile as tile
from concourse import mybir
from concourse.masks import make_identity
from concourse.bass_utils import run_bass_kernel_spmd

V = 50000
NCORES = 8
VC = V // NCORES          # 6250 owned vertices per core
P = 128
NVT = (VC + P - 1) // P   # 49 vertex tiles per core
VCP = NVT * P             # 6272 padded
NP = 22                   # neighbor slots (last = -deg*verts fold slot)
MAXNV = 8                 # tiles per chunk (8*16 = 128 partitions)


def make_chunks(nbt):
    """Chunk 0 holds the tier-B (overflow) tiles so the extra overflow reduce
    runs during pipeline fill; taper the first chunks for fast pipeline fill."""
    ch = [max(nbt, 2), 6]
    rem = NVT - sum(ch)
    while rem > MAXNV:
        ch.append(MAXNV)
        rem -= MAXNV
    if rem:
        ch.append(rem)
    assert sum(ch) == NVT and all(1 <= x <= MAXNV for x in ch)
    return ch


BF = mybir.dt.float16
BF_NP = np.float16

LAST_RESULTS = None       # BassKernelResults of the most recent run (for test.py)


def build(nc: bass.Bass, NBT: int, NPB: int, CHUNKS):
    dt = mybir.dt
    xps = [
        nc.dram_tensor(f"xp{c}", [P, nv * 3 * NP], BF, kind="ExternalInput")
        for c, nv in enumerate(CHUNKS)
    ]
    auxs = [
        nc.dram_tensor(f"aux{c}", [P, nv * 9], BF, kind="ExternalInput")
        for c, nv in enumerate(CHUNKS)
    ]
    xpb = (
        nc.dram_tensor("xpb", [P, NBT * 3 * NPB], BF, kind="ExternalInput")
        if NBT > 0
        else None
    )
    wr = nc.dram_tensor("wr", [P, 512], BF, kind="ExternalInput")
    out = nc.dram_tensor("out", [P, NVT * P], dt.float16, kind="ExternalOutput")

    with tile.TileContext(nc) as tc:
        with (
            tc.tile_pool(name="c", bufs=1) as cpool,
            tc.tile_pool(name="x", bufs=5) as xpool,
            tc.tile_pool(name="w", bufs=4) as wpool,
            tc.tile_pool(name="pt", bufs=2, space="PSUM") as pst,
            tc.tile_pool(name="pg", bufs=2, space="PSUM") as psg,
        ):
            outsb = cpool.tile([P, NVT * P], dt.float16)
            ident = cpool.tile([P, P], BF)
            with tc.high_priority():
                make_identity(nc, ident[:])
            w_t = cpool.tile([P, 512], BF)
            nc.scalar.dma_start(out=w_t[:], in_=wr[:])
            xb = None
            if NBT > 0:
                xb = cpool.tile([P, NBT * 3 * NPB], BF, tag="xb")
                nc.scalar.dma_start(out=xb[:], in_=xpb[:])
            # persistent u tiles (4-deep rotation); bias slot 9 = 1, 10:12 = 0
            u_bufs = []
            for s in range(4):
                ub = cpool.tile([P, MAXNV * 16], BF, tag=f"u{s}")
                nc.vector.memset(ub[:], 0.0)
                nc.vector.memset(
                    ub[:].rearrange("p (v s) -> p v s", s=16)[:, :, 9:10], 1.0
                )
                u_bufs.append(ub)

            NCH = len(CHUNKS)
            vstart = [0]
            for nv in CHUNKS:
                vstart.append(vstart[-1] + nv)
            state = [None] * NCH  # per-chunk (u, uT, pg)

            def stage_load_rfm(c):
                nv = CHUNKS[c]
                xt = xpool.tile([P, MAXNV * 3 * NP], BF, tag="xt")
                nc.sync.dma_start(
                    out=xt[:, : nv * 3 * NP], in_=xps[c][:]
                )
                aux_t = xpool.tile([P, MAXNV * 9], BF, tag="aux")
                nc.scalar.dma_start(out=aux_t[:, : nv * 9], in_=auxs[c][:])
                aux9 = aux_t[:, : nv * 9].rearrange("p (v f) -> p v f", f=9)
                xv = xt[:, : nv * 3 * NP].rearrange(
                    "p (v c n) -> p v c n", v=nv, c=3, n=NP
                )
                # t = sum over slots (one slot holds -deg*verts)
                t = wpool.tile([P, MAXNV * 3], BF, tag="t")
                with nc.allow_low_precision(reason="fp16 neighbor sums"):
                    nc.vector.tensor_reduce(
                        out=t[:, : nv * 3], in_=xv,
                        axis=mybir.AxisListType.X,
                        op=mybir.AluOpType.add,
                    )
                if NBT > 0 and c == 0:
                    # overflow slots of high-degree verts (the first NBT v-tiles)
                    tB = cpool.tile([P, NBT * 3], BF, tag="tB")
                    with nc.allow_low_precision(reason="fp16 neighbor sums"):
                        nc.vector.tensor_reduce(
                            out=tB[:],
                            in_=xb[:].rearrange(
                                "p (v c n) -> p v c n", v=NBT, c=3, n=NPB
                            ),
                            axis=mybir.AxisListType.X,
                            op=mybir.AluOpType.add,
                        )
                    nc.vector.tensor_tensor(
                        out=t[:, : NBT * 3],
                        in0=t[:, : NBT * 3],
                        in1=tB[:],
                        op=mybir.AluOpType.add,
                    )

                # u[p, v, j*3+k] = t[p,v,j]*lrf[p,v,j*3+k] broadcast mul (Pool)
                u = u_bufs[c % 4]
                u9 = u[:, : nv * 16].rearrange("p (v s) -> p v s", s=16)[
                    :, :, 0:9
                ].rearrange("p v (k j) -> p v k j", k=3, j=3)
                t4 = t[:, : nv * 3].rearrange("p (v c) -> p v c", c=3).unsqueeze(2)
                nc.gpsimd.tensor_tensor(
                    out=u9,
                    in0=t4.to_broadcast([P, nv, 3, 3]),
                    in1=aux9.rearrange("p v (k j) -> p v k j", k=3, j=3),
                    op=mybir.AluOpType.mult,
                )
                state[c] = [u, None, None]

            def stage_tu(c):
                nv = CHUNKS[c]
                cw = nv * 16
                u = state[c][0]
                pt = pst.tile([P, P], BF, tag="pt")
                nc.tensor.transpose(
                    out=pt[:cw, :], in_=u[:, :cw], identity=ident[:]
                )
                uT = wpool.tile([P, P], BF, tag="uT")
                nc.scalar.copy(out=uT[:cw, :], in_=pt[:cw, :])
                state[c][1] = uT

            def stage_gemm(c):
                nv = CHUNKS[c]
                uT = state[c][1]
                pg = psg.tile([P, MAXNV * P], dt.float32, tag="pg")
                g = 0
                while g < nv:
                    ng = min(4, nv - g)
                    rb = 16 * g
                    nc.tensor.matmul(
                        out=pg[:, g * P : (g + ng) * P],
                        lhsT=uT[rb : rb + 16 * ng, :],
                        rhs=w_t[rb : rb + 16 * ng, : ng * P],
                        start=True,
                        stop=True,
                    )
                    g += ng
                state[c][2] = pg

            def stage_drain_store(c):
                nv = CHUNKS[c]
                ow = nv * P
                olo = vstart[c] * P
                pg = state[c][2]
                # split drain ~3/8 DVE : 5/8 Act
                nsp = (3 * ow // 8) // P * P
                if nsp == 0:
                    nsp = min(P, ow)
                nc.vector.tensor_copy(
                    out=outsb[:, olo : olo + nsp], in_=pg[:, :nsp]
                )
                if ow > nsp:
                    nc.scalar.copy(
                        out=outsb[:, olo + nsp : olo + ow], in_=pg[:, nsp:ow]
                    )
                nc.sync.dma_start(
                    out=out[:, olo : olo + ow], in_=outsb[:, olo : olo + ow]
                )

            # software-pipelined emission: reduce/mult 2 chunks ahead of the
            # GEMM, transpose/uT 1 ahead, drain 1 behind
            for i in range(NCH + 3):
                if i < NCH:
                    stage_load_rfm(i)
                if 0 <= i - 1 < NCH:
                    stage_tu(i - 1)
                if 0 <= i - 2 < NCH:
                    stage_gemm(i - 2)
                if 0 <= i - 3 < NCH:
                    stage_drain_store(i - 3)
    return nc


def _host_prep(verts, edges, lrf, W, b):
    vb = np.asarray(verts, dtype=np.float32)
    e = np.asarray(edges).astype(np.int64)
    src = np.concatenate([e[:, 0], e[:, 1]]).astype(np.int64)
    dst = np.concatenate([e[:, 1], e[:, 0]]).astype(np.int64)

    deg = np.bincount(src, minlength=V).astype(np.int64)
    maxN = int(deg.max())
    # two-tier: main table has NP slots (last = fold); deg > NP-1 vertices are
    # remapped to the leading v-tiles and spill into the overflow table.
    CAP = NP - 1
    over = (deg > CAP).reshape(NCORES, VC)
    nB = over.sum(axis=1)
    NBT = int(np.ceil(nB.max() / P)) if maxN > CAP else 0
    NPB = max(0, ((maxN - CAP + 3) // 4) * 4)

    # per-core remap: overflow verts first (stable), then the rest
    newpos = np.empty((NCORES, VC), np.int64)
    order_c = np.empty((NCORES, VC), np.int64)
    for cc in range(NCORES):
        oc = np.concatenate([np.where(over[cc])[0], np.where(~over[cc])[0]])
        order_c[cc] = oc
        newpos[cc, oc] = np.arange(VC)

    order = np.argsort(src, kind="stable")
    src_s = src[order]
    dst_s = dst[order]
    starts = np.zeros(V + 1, np.int64)
    np.cumsum(deg, out=starts[1:])
    slot = np.arange(src_s.size, dtype=np.int64) - starts[src_s]

    c_a = src_s // VC
    il_new = newpos[c_a, src_s - c_a * VC]
    p_a = il_new % P
    v_a = il_new // P
    vals = vb[dst_s].astype(BF_NP)

    Xp = np.zeros((NCORES, P, NVT, 3, NP), BF_NP)
    inA = slot < CAP
    Xp[c_a[inA], p_a[inA], v_a[inA], :, slot[inA]] = vals[inA]
    if NBT > 0:
        XpB = np.zeros((NCORES, P, NBT, 3, NPB), BF_NP)
        inB = ~inA
        XpB[c_a[inB], p_a[inB], v_a[inB], :, slot[inB] - CAP] = vals[inB]
    else:
        XpB = np.zeros((NCORES, P, 0, 3, 0), BF_NP)

    # fold slot: -deg*verts for the owned vertex goes in the last A slot
    dv = (-deg[:, None].astype(np.float32)) * vb
    dv_pad = np.zeros((NCORES, VCP, 3), np.float32)
    for cc in range(NCORES):
        dv_pad[cc, :VC] = dv.reshape(NCORES, VC, 3)[cc][order_c[cc]]
    Xp[:, :, :, :, NP - 1] = dv_pad.reshape(NCORES, NVT, P, 3).transpose(
        0, 2, 1, 3
    ).astype(BF_NP)

    # aux per vertex: lrf(9), remapped -> [NC, P, NVT*9]
    aux_flat = np.zeros((NCORES, VCP, 9), np.float32)
    # k-major flattening: slot s = k*3+j holds lrf[:, j, k]
    lrf9 = np.ascontiguousarray(
        np.asarray(lrf, np.float32).reshape(NCORES, VC, 3, 3).transpose(0, 1, 3, 2)
    ).reshape(NCORES, VC, 9)
    for cc in range(NCORES):
        aux_flat[cc, :VC] = lrf9[cc][order_c[cc]]
    auxh = np.ascontiguousarray(
        aux_flat.reshape(NCORES, NVT, P, 9).transpose(0, 2, 1, 3)
    ).reshape(NCORES, P, NVT * 9).astype(BF_NP)

    Wf = np.asarray(W, np.float32)
    W16 = np.zeros((16, P), np.float32)
    for s in range(9):
        W16[s, :] = Wf[:, s // 3]   # k-major: slot s = k*3+j -> k = s//3
    W16[9, :] = maxN * np.asarray(b, np.float32)
    # Block-diagonal [128, 512]: 4 column blocks of W16, replicated in both
    # 64-row halves so matmuls can anchor at partition 0 or 64.
    half = np.zeros((64, 512), np.float32)
    for q in range(4):
        half[16 * q : 16 * q + 16, 128 * q : 128 * q + 128] = W16
    Wr = np.ascontiguousarray(np.vstack([half, half])).astype(BF_NP)

    CH = make_chunks(NBT)
    in_maps = []
    for c in range(NCORES):
        xpf = Xp[c].reshape(P, NVT, 3 * NP)
        auxf = auxh[c].reshape(P, NVT, 9)
        m = {"wr": Wr}
        vlo = 0
        for ci, nv in enumerate(CH):
            m[f"xp{ci}"] = np.ascontiguousarray(
                xpf[:, vlo : vlo + nv].reshape(P, nv * 3 * NP)
            )
            m[f"aux{ci}"] = np.ascontiguousarray(
                auxf[:, vlo : vlo + nv].reshape(P, nv * 9)
            )
            vlo += nv
        if NBT > 0:
            m["xpb"] = np.ascontiguousarray(XpB[c].reshape(P, NBT * 3 * NPB))
        in_maps.append(m)
    return in_maps, NBT, NPB, CH, order_c


def kernel(verts, edges, lrf, W, b):
    global LAST_RESULTS
    in_maps, NBT, NPB, CH, order_c = _host_prep(verts, edges, lrf, W, b)

    nc = bacc.Bacc()
    build(nc, NBT, NPB, CH)
    nc.finalize()

    trace = os.environ.get("KBENCH_TRACE") == "1"
    res = run_bass_kernel_spmd(
        nc, in_maps, core_ids=list(range(NCORES)), trace=trace
    )
    LAST_RESULTS = res

    full = np.empty((V, 128), np.float32)
    for c in range(NCORES):
        o = (
            res.results[c]["out"].astype(np.float32)
            .reshape(P, NVT, P).transpose(1, 0, 2).reshape(VCP, P)[:VC]
        )
        blk = full[c * VC : (c + 1) * VC]
        blk[order_c[c]] = o
    return full


# revision 38
# speedup vs baseline: 1.0004x; 1.0004x over previous
"""LRFGraphConv Trainium2 kernel.

Math: for each vertex i with neighbors N(i) (directed edge list, src=center):
    out[i] = ((sum_{j in N(i)} verts[j] - deg_i * verts[i]) @ lrf[i]) @ W.T + maxN * b

The neighbor-sum commutes with the per-center rotation and GEMM, so the
per-edge work collapses to a segment-sum of neighbor coordinates.  The
rotation and GEMM fuse into a single tensor-engine contraction over the 9
(j,k) pairs of u[i,(j,k)] = t[i,j]*lrf[i,j,k] against Wrep[(j,k),n] = W[n,k],
plus a constant-1 row carrying the maxN*b bias.  u uses 16 slots per vertex
(9 real + bias + 6 pad) so GEMM halves can anchor at partition 0/64.

Sharding: vertices are partitioned contiguously across 8 cores (6250 each).
The host buckets directed edges by owner of src, builds a per-core padded
neighbor table (NP slots, zero padded), and gathers the halo neighbor
coordinates into it (the "halo exchange" done at shard time).  Each core runs
the same NEFF on its own shard over pipelined chunks of up to 8 vertex
tiles:
  DVE:    20-slot reduce + part of the PSUM drain
  Pool:   u = t*lrf broadcast multiply
  PE:     transpose + GEMM (fp16)
  Act:    uT copy + the rest of the PSUM drain
  DMA issue: Sync (neighbor chunks + stores), Act (aux/w/overflow)
No collectives.
"""

import os
import sys

sys.path.insert(0, "/opt/trn_rl_repo")

import numpy as np
import ml_dtypes

import concourse.bass as bass
import concourse.bacc as bacc
import concourse.tile as tile
from concourse import mybir
from concourse.masks import make_identity
from concourse.bass_utils import run_bass_kernel_spmd

V = 50000
NCORES = 8
VC = V // NCORES          # 6250 owned vertices per core
P = 128
NVT = (VC + P - 1) // P   # 49 vertex tiles per core
VCP = NVT * P             # 6272 padded
NP = 22                   # neighbor slots (last = -deg*verts fold slot)
MAXNV = 8                 # tiles per chunk (8*16 = 128 partitions)


def make_chunks(nbt):
    """Chunk 0 holds the tier-B (overflow) tiles so the extra overflow reduce
    runs during pipeline fill; taper the first chunks for fast pipeline fill."""
    ch = [max(nbt, 2), 4, 6]
    rem = NVT - sum(ch)
    while rem > MAXNV:
        ch.append(MAXNV)
        rem -= MAXNV
    if rem:
        ch.append(rem)
    assert sum(ch) == NVT and all(1 <= x <= MAXNV for x in ch)
    return ch


BF = mybir.dt.float16
BF_NP = np.float16

LAST_RESULTS = None       # BassKernelResults of the most recent run (for test.py)


def build(nc: bass.Bass, NBT: int, NPB: int, CHUNKS):
    dt = mybir.dt
    FOLD = 10             # slots folded by Pool before the DVE reduce
    NR = NP - FOLD        # slots left for the DVE reduce
    xps = [
        nc.dram_tensor(f"xp{c}", [P, nv * 3 * NP], BF, kind="ExternalInput")
        for c, nv in enumerate(CHUNKS)
    ]
    auxs = [
        nc.dram_tensor(f"aux{c}", [P, nv * 9], BF, kind="ExternalInput")
        for c, nv in enumerate(CHUNKS)
    ]
    xpb = (
        nc.dram_tensor("xpb", [P, NBT * 3 * NPB], BF, kind="ExternalInput")
        if NBT > 0
        else None
    )
    wr = nc.dram_tensor("wr", [P, 512], BF, kind="ExternalInput")
    out = nc.dram_tensor("out", [P, NVT * P], dt.float16, kind="ExternalOutput")

    with tile.TileContext(nc) as tc:
        with (
            tc.tile_pool(name="c", bufs=1) as cpool,
            tc.tile_pool(name="x", bufs=5) as xpool,
            tc.tile_pool(name="w", bufs=4) as wpool,
            tc.tile_pool(name="pt", bufs=2, space="PSUM") as pst,
            tc.tile_pool(name="pg", bufs=3, space="PSUM") as psg,
        ):
            outsb = cpool.tile([P, NVT * P], dt.float16)
            ident = cpool.tile([P, P], BF)
            with tc.high_priority():
                make_identity(nc, ident[:])
            w_t = cpool.tile([P, 512], BF)
            nc.scalar.dma_start(out=w_t[:], in_=wr[:])
            xb = None
            if NBT > 0:
                xb = cpool.tile([P, NBT * 3 * NPB], BF, tag="xb")
                nc.scalar.dma_start(out=xb[:], in_=xpb[:])
            # persistent u tiles (4-deep rotation); bias slot 9 = 1, 10:12 = 0
            u_bufs = []
            for s in range(4):
                ub = cpool.tile([P, MAXNV * 16], BF, tag=f"u{s}")
                nc.vector.memset(ub[:], 0.0)
                nc.vector.memset(
                    ub[:].rearrange("p (v s) -> p v s", s=16)[:, :, 9:10], 1.0
                )
                u_bufs.append(ub)

            NCH = len(CHUNKS)
            vstart = [0]
            for nv in CHUNKS:
                vstart.append(vstart[-1] + nv)
            state = [None] * NCH  # per-chunk (u, uT, pg)

            def stage_load_rfm(c):
                nv = CHUNKS[c]
                xt = xpool.tile([P, MAXNV * 3 * NP], BF, tag="xt")
                nc.sync.dma_start(
                    out=xt[:, : nv * 3 * NP], in_=xps[c][:]
                )
                aux_t = xpool.tile([P, MAXNV * 9], BF, tag="aux")
                nc.scalar.dma_start(out=aux_t[:, : nv * 9], in_=auxs[c][:])
                aux9 = aux_t[:, : nv * 9].rearrange("p (v f) -> p v f", f=9)
                xv = xt[:, : nv * 3 * NP].rearrange(
                    "p (v c n) -> p v c n", v=nv, c=3, n=NP
                )
                # Pool folds the top FOLD slots into slots NR-FOLD:NR in place
                with nc.allow_low_precision(reason="fp16 neighbor sums"):
                    nc.gpsimd.tensor_tensor(
                        out=xv[:, :, :, NR - FOLD : NR],
                        in0=xv[:, :, :, NR - FOLD : NR],
                        in1=xv[:, :, :, NR:NP],
                        op=mybir.AluOpType.add,
                    )
                # t = sum over remaining NR slots (one holds -deg*verts)
                t = wpool.tile([P, MAXNV * 3], BF, tag="t")
                with nc.allow_low_precision(reason="fp16 neighbor sums"):
                    nc.vector.tensor_reduce(
                        out=t[:, : nv * 3], in_=xv[:, :, :, :NR],
                        axis=mybir.AxisListType.X,
                        op=mybir.AluOpType.add,
                    )
                if NBT > 0 and c == 0:
                    # overflow slots of high-degree verts (the first NBT v-tiles)
                    tB = cpool.tile([P, NBT * 3], BF, tag="tB")
                    with nc.allow_low_precision(reason="fp16 neighbor sums"):
                        nc.vector.tensor_reduce(
                            out=tB[:],
                            in_=xb[:].rearrange(
                                "p (v c n) -> p v c n", v=NBT, c=3, n=NPB
                            ),
                            axis=mybir.AxisListType.X,
                            op=mybir.AluOpType.add,
                        )
                    nc.vector.tensor_tensor(
                        out=t[:, : NBT * 3],
                        in0=t[:, : NBT * 3],
                        in1=tB[:],
                        op=mybir.AluOpType.add,
                    )

                # u[p, v, j*3+k] = t[p,v,j]*lrf[p,v,j*3+k] broadcast mul (Pool)
                u = u_bufs[c % 4]
                u9 = u[:, : nv * 16].rearrange("p (v s) -> p v s", s=16)[
                    :, :, 0:9
                ].rearrange("p v (k j) -> p v k j", k=3, j=3)
                t4 = t[:, : nv * 3].rearrange("p (v c) -> p v c", c=3).unsqueeze(2)
                nc.gpsimd.tensor_tensor(
                    out=u9,
                    in0=t4.to_broadcast([P, nv, 3, 3]),
                    in1=aux9.rearrange("p v (k j) -> p v k j", k=3, j=3),
                    op=mybir.AluOpType.mult,
                )
                state[c] = [u, None, None]

            def stage_tu(c):
                nv = CHUNKS[c]
                cw = nv * 16
                u = state[c][0]
                pt = pst.tile([P, P], BF, tag="pt")
                nc.tensor.transpose(
                    out=pt[:cw, :], in_=u[:, :cw], identity=ident[:]
                )
                uT = wpool.tile([P, P], BF, tag="uT")
                nc.scalar.copy(out=uT[:cw, :], in_=pt[:cw, :])
                state[c][1] = uT

            def stage_gemm(c):
                nv = CHUNKS[c]
                uT = state[c][1]
                pg = psg.tile([P, MAXNV * P], dt.float32, tag="pg")
                g = 0
                while g < nv:
                    ng = min(4, nv - g)
                    rb = 16 * g
                    nc.tensor.matmul(
                        out=pg[:, g * P : (g + ng) * P],
                        lhsT=uT[rb : rb + 16 * ng, :],
                        rhs=w_t[rb : rb + 16 * ng, : ng * P],
                        start=True,
                        stop=True,
                    )
                    g += ng
                state[c][2] = pg

            def stage_drain_store(c):
                nv = CHUNKS[c]
                ow = nv * P
                olo = vstart[c] * P
                pg = state[c][2]
                # split drain ~3/8 DVE : 5/8 Act
                nsp = (3 * ow // 8) // P * P
                if nsp == 0:
                    nsp = min(P, ow)
                nc.vector.tensor_copy(
                    out=outsb[:, olo : olo + nsp], in_=pg[:, :nsp]
                )
                if ow > nsp:
                    nc.scalar.copy(
                        out=outsb[:, olo + nsp : olo + ow], in_=pg[:, nsp:ow]
                    )
                nc.sync.dma_start(
                    out=out[:, olo : olo + ow], in_=outsb[:, olo : olo + ow]
                )

            # software-pipelined emission: reduce/mult 2 chunks ahead of the
            # GEMM, transpose/uT 1 ahead, drain 1 behind
            for i in range(NCH + 3):
                if i < NCH:
                    stage_load_rfm(i)
                if 0 <= i - 1 < NCH:
                    stage_tu(i - 1)
                if 0 <= i - 2 < NCH:
                    stage_gemm(i - 2)
                if 0 <= i - 3 < NCH:
                    stage_drain_store(i - 3)
    return nc


def _host_prep(verts, edges, lrf, W, b):
    vb = np.asarray(verts, dtype=np.float32)
    e = np.asarray(edges).astype(np.int64)
    src = np.concatenate([e[:, 0], e[:, 1]]).astype(np.int64)
    dst = np.concatenate([e[:, 1], e[:, 0]]).astype(np.int64)

    deg = np.bincount(src, minlength=V).astype(np.int64)
    maxN = int(deg.max())
    # two-tier: main table has NP slots (last = fold); deg > NP-1 vertices are
    # remapped to the leading v-tiles and spill into the overflow table.
    CAP = NP - 1
    over = (deg > CAP).reshape(NCORES, VC)
    nB = over.sum(axis=1)
    NBT = int(np.ceil(nB.max() / P)) if maxN > CAP else 0
    NPB = max(0, ((maxN - CAP + 3) // 4) * 4)

    # per-core remap: overflow verts first (stable), then the rest
    newpos = np.empty((NCORES, VC), np.int64)
    order_c = np.empty((NCORES, VC), np.int64)
    for cc in range(NCORES):
        oc = np.concatenate([np.where(over[cc])[0], np.where(~over[cc])[0]])
        order_c[cc] = oc
        newpos[cc, oc] = np.arange(VC)

    order = np.argsort(src, kind="stable")
    src_s = src[order]
    dst_s = dst[order]
    starts = np.zeros(V + 1, np.int64)
    np.cumsum(deg, out=starts[1:])
    slot = np.arange(src_s.size, dtype=np.int64) - starts[src_s]

    c_a = src_s // VC
    il_new = newpos[c_a, src_s - c_a * VC]
    p_a = il_new % P
    v_a = il_new // P
    vals = vb[dst_s].astype(BF_NP)

    Xp = np.zeros((NCORES, P, NVT, 3, NP), BF_NP)
    inA = slot < CAP
    Xp[c_a[inA], p_a[inA], v_a[inA], :, slot[inA]] = vals[inA]
    if NBT > 0:
        XpB = np.zeros((NCORES, P, NBT, 3, NPB), BF_NP)
        inB = ~inA
        XpB[c_a[inB], p_a[inB], v_a[inB], :, slot[inB] - CAP] = vals[inB]
    else:
        XpB = np.zeros((NCORES, P, 0, 3, 0), BF_NP)

    # fold slot: -deg*verts for the owned vertex goes in the last A slot
    dv = (-deg[:, None].astype(np.float32)) * vb
    dv_pad = np.zeros((NCORES, VCP, 3), np.float32)
    for cc in range(NCORES):
        dv_pad[cc, :VC] = dv.reshape(NCORES, VC, 3)[cc][order_c[cc]]
    Xp[:, :, :, :, NP - 1] = dv_pad.reshape(NCORES, NVT, P, 3).transpose(
        0, 2, 1, 3
    ).astype(BF_NP)

    # aux per vertex: lrf(9), remapped -> [NC, P, NVT*9]
    aux_flat = np.zeros((NCORES, VCP, 9), np.float32)
    # k-major flattening: slot s = k*3+j holds lrf[:, j, k]
    lrf9 = np.ascontiguousarray(
        np.asarray(lrf, np.float32).reshape(NCORES, VC, 3, 3).transpose(0, 1, 3, 2)
    ).reshape(NCORES, VC, 9)
    for cc in range(NCORES):
        aux_flat[cc, :VC] = lrf9[cc][order_c[cc]]
    auxh = np.ascontiguousarray(
        aux_flat.reshape(NCORES, NVT, P, 9).transpose(0, 2, 1, 3)
    ).reshape(NCORES, P, NVT * 9).astype(BF_NP)

    Wf = np.asarray(W, np.float32)
    W16 = np.zeros((16, P), np.float32)
    for s in range(9):
        W16[s, :] = Wf[:, s // 3]   # k-major: slot s = k*3+j -> k = s//3
    W16[9, :] = maxN * np.asarray(b, np.float32)
    # Block-diagonal [128, 512]: 4 column blocks of W16, replicated in both
    # 64-row halves so matmuls can anchor at partition 0 or 64.
    half = np.zeros((64, 512), np.float32)
    for q in range(4):
        half[16 * q : 16 * q + 16, 128 * q : 128 * q + 128] = W16
    Wr = np.ascontiguousarray(np.vstack([half, half])).astype(BF_NP)

    CH = make_chunks(NBT)
    in_maps = []
    for c in range(NCORES):
        xpf = Xp[c].reshape(P, NVT, 3 * NP)
        auxf = auxh[c].reshape(P, NVT, 9)
        m = {"wr": Wr}
        vlo = 0
        for ci, nv in enumerate(CH):
            m[f"xp{ci}"] = np.ascontiguousarray(
                xpf[:, vlo : vlo + nv].reshape(P, nv * 3 * NP)
            )
            m[f"aux{ci}"] = np.ascontiguousarray(
                auxf[:, vlo : vlo + nv].reshape(P, nv * 9)
            )
            vlo += nv
        if NBT > 0:
            m["xpb"] = np.ascontiguousarray(XpB[c].reshape(P, NBT * 3 * NPB))
        in_maps.append(m)
    return in_maps, NBT, NPB, CH, order_c


def kernel(verts, edges, lrf, W, b):
    global LAST_RESULTS
    in_maps, NBT, NPB, CH, order_c = _host_prep(verts, edges, lrf, W, b)

    nc = bacc.Bacc()
    build(nc, NBT, NPB, CH)
    nc.finalize()

    trace = os.environ.get("KBENCH_TRACE") == "1"
    res = run_bass_kernel_spmd(
        nc, in_maps, core_ids=list(range(NCORES)), trace=trace
    )
    LAST_RESULTS = res

    full = np.empty((V, 128), np.float32)
    for c in range(NCORES):
        o = (
            res.results[c]["out"].astype(np.float32)
            .reshape(P, NVT, P).transpose(1, 0, 2).reshape(VCP, P)[:VC]
        )
        blk = full[c * VC : (c + 1) * VC]
        blk[order_c[c]] = o
    return full


# revision 42
# speedup vs baseline: 1.1324x; 1.1320x over previous
"""LRFGraphConv Trainium2 kernel.

Math: for each vertex i with neighbors N(i) (directed edge list, src=center):
    out[i] = ((sum_{j in N(i)} verts[j] - deg_i * verts[i]) @ lrf[i]) @ W.T + maxN * b

The neighbor-sum commutes with the per-center rotation and GEMM, so the
per-edge work collapses to a segment-sum of neighbor coordinates.  The
rotation and GEMM fuse into a single tensor-engine contraction over the 9
(j,k) pairs of u[i,(j,k)] = t[i,j]*lrf[i,j,k] against Wrep[(j,k),n] = W[n,k],
plus a constant-1 row carrying the maxN*b bias.  u uses 16 slots per vertex
(9 real + bias + 6 pad) so GEMM halves can anchor at partition 0/64.

Sharding: vertices are partitioned contiguously across 8 cores (6250 each).
The host buckets directed edges by owner of src, builds a per-core padded
neighbor table (NP slots, zero padded), and gathers the halo neighbor
coordinates into it (the "halo exchange" done at shard time).  Each core runs
the same NEFF on its own shard over pipelined chunks of up to 8 vertex
tiles:
  DVE:    20-slot reduce + part of the PSUM drain
  Pool:   u = t*lrf broadcast multiply
  PE:     transpose + GEMM (fp16)
  Act:    uT copy + the rest of the PSUM drain
  DMA issue: Sync (neighbor chunks + stores), Act (aux/w/overflow)
No collectives.
"""

import os
import sys

sys.path.insert(0, "/opt/trn_rl_repo")

import numpy as np
import ml_dtypes

import concourse.bass as bass
import concourse.bacc as bacc
import concourse.tile as tile
from concourse import mybir
from concourse.masks import make_identity
from concourse.bass_utils import run_bass_kernel_spmd

V = 50000
NCORES = 8
VC = V // NCORES          # 6250 owned vertices per core
P = 128
NVT = (VC + P - 1) // P   # 49 vertex tiles per core
VCP = NVT * P             # 6272 padded
NP = 22                   # neighbor slots (last = -deg*verts fold slot)
MAXNV = 8                 # tiles per chunk (8*16 = 128 partitions)


def make_chunks(nbt):
    """Chunk 0 holds the tier-B (overflow) tiles so the extra overflow reduce
    runs during pipeline fill; taper the first chunks for fast pipeline fill."""
    ch = [max(nbt, 2), 4, 6]
    rem = NVT - sum(ch)
    while rem > MAXNV:
        ch.append(MAXNV)
        rem -= MAXNV
    if rem:
        ch.append(rem)
    assert sum(ch) == NVT and all(1 <= x <= MAXNV for x in ch)
    return ch


BF = mybir.dt.float16
BF_NP = np.float16

LAST_RESULTS = None       # BassKernelResults of the most recent run (for test.py)


def build(nc: bass.Bass, NBT: int, NPB: int, CHUNKS):
    dt = mybir.dt
    FOLD = 10             # slots folded by Pool before the DVE reduce
    NR = NP - FOLD        # slots left for the DVE reduce
    xps = [
        nc.dram_tensor(f"xp{c}", [P, nv * 3 * NP], BF, kind="ExternalInput")
        for c, nv in enumerate(CHUNKS)
    ]
    aux = nc.dram_tensor("aux", [P, NVT * 9], BF, kind="ExternalInput")
    xpb = (
        nc.dram_tensor("xpb", [P, NBT * 3 * NPB], BF, kind="ExternalInput")
        if NBT > 0
        else None
    )
    wr = nc.dram_tensor("wr", [P, 512], BF, kind="ExternalInput")
    out = nc.dram_tensor("out", [P, NVT * P], dt.float16, kind="ExternalOutput")

    with tile.TileContext(nc) as tc:
        with (
            tc.tile_pool(name="c", bufs=1) as cpool,
            tc.tile_pool(name="x", bufs=5) as xpool,
            tc.tile_pool(name="w", bufs=4) as wpool,
            tc.tile_pool(name="pt", bufs=2, space="PSUM") as pst,
            tc.tile_pool(name="pg", bufs=3, space="PSUM") as psg,
        ):
            outsb = cpool.tile([P, NVT * P], dt.float16)
            ident = cpool.tile([P, P], BF)
            with tc.high_priority():
                make_identity(nc, ident[:])
            # aux in two parts: a small early slice so chunk 0-2 multiplies
            # start fast, then the bulk
            AUX_SPLIT = (CHUNKS[0] + CHUNKS[1] + CHUNKS[2]) * 9
            aux_t = cpool.tile([P, NVT * 9], BF)
            nc.scalar.dma_start(out=aux_t[:, :AUX_SPLIT], in_=aux[:, :AUX_SPLIT])
            nc.scalar.dma_start(out=aux_t[:, AUX_SPLIT:], in_=aux[:, AUX_SPLIT:])
            auxfull = aux_t[:].rearrange("p (v f) -> p v f", f=9)
            w_t = cpool.tile([P, 512], BF)
            nc.scalar.dma_start(out=w_t[:], in_=wr[:])
            xb = None
            if NBT > 0:
                xb = cpool.tile([P, NBT * 3 * NPB], BF, tag="xb")
                nc.scalar.dma_start(out=xb[:], in_=xpb[:])
            # persistent u tiles (4-deep rotation); bias slot 9 = 1, 10:12 = 0
            u_bufs = []
            for s in range(4):
                ub = cpool.tile([P, MAXNV * 16], BF, tag=f"u{s}")
                nc.vector.memset(ub[:], 0.0)
                nc.vector.memset(
                    ub[:].rearrange("p (v s) -> p v s", s=16)[:, :, 9:10], 1.0
                )
                u_bufs.append(ub)

            NCH = len(CHUNKS)
            vstart = [0]
            for nv in CHUNKS:
                vstart.append(vstart[-1] + nv)
            state = [None] * NCH  # per-chunk (u, uT, pg)

            def stage_load_rfm(c):
                nv = CHUNKS[c]
                xt = xpool.tile([P, MAXNV * 3 * NP], BF, tag="xt")
                nc.sync.dma_start(
                    out=xt[:, : nv * 3 * NP], in_=xps[c][:]
                )
                vlo = vstart[c]
                aux9 = auxfull[:, vlo : vlo + nv, :]
                xv = xt[:, : nv * 3 * NP].rearrange(
                    "p (v c n) -> p v c n", v=nv, c=3, n=NP
                )
                # Pool folds the top FOLD slots into slots NR-FOLD:NR in place
                with nc.allow_low_precision(reason="fp16 neighbor sums"):
                    nc.gpsimd.tensor_tensor(
                        out=xv[:, :, :, NR - FOLD : NR],
                        in0=xv[:, :, :, NR - FOLD : NR],
                        in1=xv[:, :, :, NR:NP],
                        op=mybir.AluOpType.add,
                    )
                # t = sum over remaining NR slots (one holds -deg*verts)
                t = wpool.tile([P, MAXNV * 3], BF, tag="t")
                with nc.allow_low_precision(reason="fp16 neighbor sums"):
                    nc.vector.tensor_reduce(
                        out=t[:, : nv * 3], in_=xv[:, :, :, :NR],
                        axis=mybir.AxisListType.X,
                        op=mybir.AluOpType.add,
                    )
                if NBT > 0 and c == 0:
                    # overflow slots of high-degree verts (the first NBT v-tiles)
                    tB = cpool.tile([P, NBT * 3], BF, tag="tB")
                    with nc.allow_low_precision(reason="fp16 neighbor sums"):
                        nc.vector.tensor_reduce(
                            out=tB[:],
                            in_=xb[:].rearrange(
                                "p (v c n) -> p v c n", v=NBT, c=3, n=NPB
                            ),
                            axis=mybir.AxisListType.X,
                            op=mybir.AluOpType.add,
                        )
                    nc.vector.tensor_tensor(
                        out=t[:, : NBT * 3],
                        in0=t[:, : NBT * 3],
                        in1=tB[:],
                        op=mybir.AluOpType.add,
                    )

                # u[p, v, j*3+k] = t[p,v,j]*lrf[p,v,j*3+k] broadcast mul (Pool)
                u = u_bufs[c % 4]
                u9 = u[:, : nv * 16].rearrange("p (v s) -> p v s", s=16)[
                    :, :, 0:9
                ].rearrange("p v (k j) -> p v k j", k=3, j=3)
                t4 = t[:, : nv * 3].rearrange("p (v c) -> p v c", c=3).unsqueeze(2)
                nc.gpsimd.tensor_tensor(
                    out=u9,
                    in0=t4.to_broadcast([P, nv, 3, 3]),
                    in1=aux9.rearrange("p v (k j) -> p v k j", k=3, j=3),
                    op=mybir.AluOpType.mult,
                )
                state[c] = [u, None, None]

            def stage_tu(c):
                nv = CHUNKS[c]
                cw = nv * 16
                u = state[c][0]
                pt = pst.tile([P, P], BF, tag="pt")
                nc.tensor.transpose(
                    out=pt[:cw, :], in_=u[:, :cw], identity=ident[:]
                )
                uT = wpool.tile([P, P], BF, tag="uT")
                nc.scalar.copy(out=uT[:cw, :], in_=pt[:cw, :])
                state[c][1] = uT

            def stage_gemm(c):
                nv = CHUNKS[c]
                uT = state[c][1]
                pg = psg.tile([P, MAXNV * P], dt.float32, tag="pg")
                g = 0
                while g < nv:
                    ng = min(4, nv - g)
                    rb = 16 * g
                    nc.tensor.matmul(
                        out=pg[:, g * P : (g + ng) * P],
                        lhsT=uT[rb : rb + 16 * ng, :],
                        rhs=w_t[rb : rb + 16 * ng, : ng * P],
                        start=True,
                        stop=True,
                    )
                    g += ng
                state[c][2] = pg

            def stage_drain_store(c):
                nv = CHUNKS[c]
                ow = nv * P
                olo = vstart[c] * P
                pg = state[c][2]
                # split drain ~3/8 DVE : 5/8 Act
                nsp = (3 * ow // 8) // P * P
                if nsp == 0:
                    nsp = min(P, ow)
                nc.vector.tensor_copy(
                    out=outsb[:, olo : olo + nsp], in_=pg[:, :nsp]
                )
                if ow > nsp:
                    nc.scalar.copy(
                        out=outsb[:, olo + nsp : olo + ow], in_=pg[:, nsp:ow]
                    )
                nc.sync.dma_start(
                    out=out[:, olo : olo + ow], in_=outsb[:, olo : olo + ow]
                )

            # software-pipelined emission: reduce/mult 2 chunks ahead of the
            # GEMM, transpose/uT 1 ahead, drain 1 behind
            for i in range(NCH + 3):
                if i < NCH:
                    stage_load_rfm(i)
                if 0 <= i - 1 < NCH:
                    stage_tu(i - 1)
                if 0 <= i - 2 < NCH:
                    stage_gemm(i - 2)
                if 0 <= i - 3 < NCH:
                    stage_drain_store(i - 3)
    return nc


def _host_prep(verts, edges, lrf, W, b):
    vb = np.asarray(verts, dtype=np.float32)
    e = np.asarray(edges).astype(np.int64)
    src = np.concatenate([e[:, 0], e[:, 1]]).astype(np.int64)
    dst = np.concatenate([e[:, 1], e[:, 0]]).astype(np.int64)

    deg = np.bincount(src, minlength=V).astype(np.int64)
    maxN = int(deg.max())
    # two-tier: main table has NP slots (last = fold); deg > NP-1 vertices are
    # remapped to the leading v-tiles and spill into the overflow table.
    CAP = NP - 1
    over = (deg > CAP).reshape(NCORES, VC)
    nB = over.sum(axis=1)
    NBT = int(np.ceil(nB.max() / P)) if maxN > CAP else 0
    NPB = max(0, ((maxN - CAP + 3) // 4) * 4)

    # per-core remap: overflow verts first (stable), then the rest
    newpos = np.empty((NCORES, VC), np.int64)
    order_c = np.empty((NCORES, VC), np.int64)
    for cc in range(NCORES):
        oc = np.concatenate([np.where(over[cc])[0], np.where(~over[cc])[0]])
        order_c[cc] = oc
        newpos[cc, oc] = np.arange(VC)

    order = np.argsort(src, kind="stable")
    src_s = src[order]
    dst_s = dst[order]
    starts = np.zeros(V + 1, np.int64)
    np.cumsum(deg, out=starts[1:])
    slot = np.arange(src_s.size, dtype=np.int64) - starts[src_s]

    c_a = src_s // VC
    il_new = newpos[c_a, src_s - c_a * VC]
    p_a = il_new % P
    v_a = il_new // P
    vals = vb[dst_s].astype(BF_NP)

    Xp = np.zeros((NCORES, P, NVT, 3, NP), BF_NP)
    inA = slot < CAP
    Xp[c_a[inA], p_a[inA], v_a[inA], :, slot[inA]] = vals[inA]
    if NBT > 0:
        XpB = np.zeros((NCORES, P, NBT, 3, NPB), BF_NP)
        inB = ~inA
        XpB[c_a[inB], p_a[inB], v_a[inB], :, slot[inB] - CAP] = vals[inB]
    else:
        XpB = np.zeros((NCORES, P, 0, 3, 0), BF_NP)

    # fold slot: -deg*verts for the owned vertex goes in the last A slot
    dv = (-deg[:, None].astype(np.float32)) * vb
    dv_pad = np.zeros((NCORES, VCP, 3), np.float32)
    for cc in range(NCORES):
        dv_pad[cc, :VC] = dv.reshape(NCORES, VC, 3)[cc][order_c[cc]]
    Xp[:, :, :, :, NP - 1] = dv_pad.reshape(NCORES, NVT, P, 3).transpose(
        0, 2, 1, 3
    ).astype(BF_NP)

    # aux per vertex: lrf(9), remapped -> [NC, P, NVT*9]
    aux_flat = np.zeros((NCORES, VCP, 9), np.float32)
    # k-major flattening: slot s = k*3+j holds lrf[:, j, k]
    lrf9 = np.ascontiguousarray(
        np.asarray(lrf, np.float32).reshape(NCORES, VC, 3, 3).transpose(0, 1, 3, 2)
    ).reshape(NCORES, VC, 9)
    for cc in range(NCORES):
        aux_flat[cc, :VC] = lrf9[cc][order_c[cc]]
    auxh = np.ascontiguousarray(
        aux_flat.reshape(NCORES, NVT, P, 9).transpose(0, 2, 1, 3)
    ).reshape(NCORES, P, NVT * 9).astype(BF_NP)

    Wf = np.asarray(W, np.float32)
    W16 = np.zeros((16, P), np.float32)
    for s in range(9):
        W16[s, :] = Wf[:, s // 3]   # k-major: slot s = k*3+j -> k = s//3
    W16[9, :] = maxN * np.asarray(b, np.float32)
    # Block-diagonal [128, 512]: 4 column blocks of W16, replicated in both
    # 64-row halves so matmuls can anchor at partition 0 or 64.
    half = np.zeros((64, 512), np.float32)
    for q in range(4):
        half[16 * q : 16 * q + 16, 128 * q : 128 * q + 128] = W16
    Wr = np.ascontiguousarray(np.vstack([half, half])).astype(BF_NP)

    CH = make_chunks(NBT)
    in_maps = []
    for c in range(NCORES):
        xpf = Xp[c].reshape(P, NVT, 3 * NP)
        m = {"wr": Wr, "aux": np.ascontiguousarray(auxh[c])}
        vlo = 0
        for ci, nv in enumerate(CH):
            m[f"xp{ci}"] = np.ascontiguousarray(
                xpf[:, vlo : vlo + nv].reshape(P, nv * 3 * NP)
            )
            vlo += nv
        if NBT > 0:
            m["xpb"] = np.ascontiguousarray(XpB[c].reshape(P, NBT * 3 * NPB))
        in_maps.append(m)
    return in_maps, NBT, NPB, CH, order_c


def kernel(verts, edges, lrf, W, b):
    global LAST_RESULTS
    in_maps, NBT, NPB, CH, order_c = _host_prep(verts, edges, lrf, W, b)

    nc = bacc.Bacc()
    build(nc, NBT, NPB, CH)
    nc.finalize()

    trace = os.environ.get("KBENCH_TRACE") == "1"
    res = run_bass_kernel_spmd(
        nc, in_maps, core_ids=list(range(NCORES)), trace=trace
    )
    LAST_RESULTS = res

    full = np.empty((V, 128), np.float32)
    for c in range(NCORES):
        o = (
            res.results[c]["out"].astype(np.float32)
            .reshape(P, NVT, P).transpose(1, 0, 2).reshape(VCP, P)[:VC]
        )
        blk = full[c * VC : (c + 1) * VC]
        blk[order_c[c]] = o
    return full
